# revision 46
# baseline (speedup 1.0000x reference)
"""ContrastiveLoss Trainium2 kernel (8 NeuronCores, SPMD row-sharded,
symmetry-exploiting).

Math (reference):
    f = features / ||features||_row            (L2 normalize)
    s_ij = (f_i . f_j) / T,  T = 0.1
    Z_i = sum_{j != i} exp(s_ij)
    per_row_i = (npos_i * ln(Z_i) - sum_{j in pos, j != i} s_ij) / (npos_i + eps)
    loss = mean(per_row)

Device computes ONLY the O(N^2) part: partial sums of exp(10 * f_i . f_j).
Each unordered (i, j) pair is computed exactly once across the 8 cores
(N^2/2 total work): core k owns rows [k*1024, (k+1)*1024) and computes
local columns [0, 4608) of its rotated column space:
  [0, 1024)     own (diagonal) block, self-terms masked
  [1024, 4096)  the next 3 row-blocks (d = 1, 2, 3)
  [4096, 4608)  a checkerboard half of the d = 4 block: row-half rb0-3 gets
                one 512-column half, rb4-7 the other; cores 4-7 receive the
                two halves host-swapped so the 4 quarters of each d=4 block
                pair are covered exactly once globally.
Row sums (ACT accum) give Z contributions for the core's own rows; column
sums (DoubleRow select-ones matmuls over fp8e5 exp tiles, accumulated
in-place in one PSUM bank) give the contributions to the partner rows,
which the host scatters back. All O(N*D) math (normalize, transpose,
class sums, final assembly) runs on the host in f32.
"""

import numpy as np
import ml_dtypes

TEMP_INV = 10.0  # 1/temperature
EPS = 1e-8
N, D, NCORES = 8192, 512, 8
RPC = N // NCORES        # 1024 rows per core
RT = RPC // 128          # 8 row tiles (128 rows) per core
KC = D // 128            # 4 contraction chunks of 128
GW = 1536                # psum group width (3 banks)
NB = 3                   # bands (psum groups) per row tile
LC = 5120                # local columns resident in SBUF
CC = 4096                # columns with column-sum output (local 1024..5120)
WARMUP_MMS = 40          # dummy matmuls to keep the PE busy (and the HAM
                         # clock gate warming) until the input DMA lands

# bands cover local columns [0,1536), [1536,3072), [3072,5120); band 2's
# third matmul tile is the d4 half: local col 4096 + 512*(rb//4)
BAND_ORDER = [0, 1, 2]  # band 0 first: it holds the lhsT columns
BAND_W = {0: 1536, 1: 1536, 2: 2048}
# colsum chunks per band: (psum offset, accumulator a/b, accumulator row)
BAND_CS = [((1024, "a", 0),),
           ((0, "a", 1), (512, "a", 2), (1024, "a", 3)),
           ((0, "b", 0), (512, "b", 1), (1024, "b", "d4"))]
NCS_A = 16               # colsum matmuls into accumulator a (bands 0-1)
NCS_B = 12               # colsum matmuls into accumulator b (band 2 + d4)

_prog_cache = None


def _build_program():
    import concourse.bacc as bacc
    import concourse.tile as tile
    from concourse import mybir

    f32, bf16 = mybir.dt.float32, mybir.dt.bfloat16
    fp8, fp8e5 = mybir.dt.float8e4, mybir.dt.float8e5
    A = mybir.ActivationFunctionType
    Alu = mybir.AluOpType
    X = mybir.AxisListType.X
    DR = mybir.MatmulPerfMode.DoubleRow

    nc = bacc.Bacc("TRN2", target_bir_lowering=False, debug=False,
                   num_devices=NCORES)

    # chunk-major packed transposed features, [128, KC*LC] fp8e4; each
    # 512-col chunk is one contiguous 2 KiB run per partition
    fT8d = nc.dram_tensor("fT8", [128, KC * LC], fp8, kind="ExternalInput")
    seld = nc.dram_tensor("sel8", [128, 2, 16], fp8e5, kind="ExternalInput")
    zoutd = nc.dram_tensor("zout", [128, RT * NB], f32,
                           kind="ExternalOutput")
    coutad = nc.dram_tensor("cout_a", [4, 512], f32, kind="ExternalOutput")
    coutbd = nc.dram_tensor("cout_b", [4, 512], f32, kind="ExternalOutput")

    from contextlib import ExitStack

    with tile.TileContext(nc) as tc, ExitStack() as ctx:
        singles = ctx.enter_context(tc.tile_pool(name="singles", bufs=1))
        esc8p = ctx.enter_context(tc.tile_pool(name="esc8p", bufs=6))

        # chunk-major band tiles: [128, chunk, KC, 512] so each 512-col DMA
        # chunk is one contiguous 2 KiB run per partition (1 descriptor)
        fb0 = singles.tile([128, 3, KC, 512], fp8, tag="fb0", name="fb0")
        fb1 = singles.tile([128, 3, KC, 512], fp8, tag="fb1", name="fb1")
        fb2 = singles.tile([128, 4, KC, 512], fp8, tag="fb2", name="fb2")
        fbands = [fb0, fb1, fb2]
        sel8 = singles.tile([128, 2, 16], fp8e5, tag="sel8")
        warm = singles.tile([128, 256], fp8, tag="warm")
        dumm = singles.tile([128, 256], bf16, tag="dumm")
        zacs = singles.tile([128, RT * NB], f32, tag="zacs")
        couts_a = singles.tile([4, 512], f32, tag="couts_a")
        couts_b = singles.tile([4, 512], f32, tag="couts_b")

        nc.vector.memset(warm, 0.0)

        # input DMA, chunked 512 cols at a time in compute order; band 0's
        # three chunks go out first on three queues in parallel — they gate
        # the first psum group, so they get the full HBM bandwidth. Bands
        # 1-2 are issued from gpsimd only after a tiny gate-copy that reads
        # one element of each band-0 chunk (i.e. after band 0 has landed),
        # so their transfers don't steal bandwidth from band 0.
        gate = singles.tile([1, 3, 1], fp8, tag="gate")
        nc.gpsimd.dma_start(out=sel8, in_=seld.ap())
        b0_queues = [nc.sync, nc.gpsimd, nc.scalar]
        o = 0
        for c in range(3):
            b0_queues[c].dma_start(
                out=fb0[:, c],
                in_=fT8d.ap()[:, o:o + KC * 512].rearrange(
                    "p (k w) -> p k w", k=KC))
            o += KC * 512
        nc.gpsimd.tensor_copy(out=gate, in_=fb0[0:1, :, 0, 0:1])
        for b in (1, 2):
            for c in range(BAND_W[b] // 512):
                nc.gpsimd.dma_start(
                    out=fbands[b][:, c],
                    in_=fT8d.ap()[:, o:o + KC * 512].rearrange(
                        "p (k w) -> p k w", k=KC))
                o += KC * 512

        with tc.tile_pool(name="mps", bufs=2, space="PSUM") as mpp, \
                tc.tile_pool(name="cps", bufs=1, space="PSUM") as cpp:
            # two accumulator banks for the column sums: row r of cs_a/cs_b
            # accumulates column-sum chunk r / 4+r (via select-ones weights);
            # cs_a (bands 0-1) completes early and is flushed mid-kernel
            cs_a = cpp.tile([4, 512], f32, tag="cs_a")
            cs_b = cpp.tile([4, 512], f32, tag="cs_b")
            cs_acc = {"a": cs_a, "b": cs_b}
            cs_n = {"a": 0, "b": 0}
            cs_tot = {"a": NCS_A, "b": NCS_B}
            cs_out = {"a": (couts_a, coutad), "b": (couts_b, coutbd)}

            # warm the PE + trigger the ACT exp table load while DMA streams
            wps = mpp.tile([128, GW], f32, tag="ps")
            for w in range(WARMUP_MMS):
                nc.tensor.matmul(wps[:, :128], lhsT=warm[:, :128],
                                 rhs=warm[:, 128:], start=True, stop=True)
            nc.scalar.activation(out=dumm, in_=warm, func=A.Exp,
                                 scale=TEMP_INV)

            esc_pair = [None]
            pend = []  # deferred colsums: (ready_gidx, band, pair, esc tile)
            cs_q = []  # colsum closures, popped one per main matmul slot

            def one_colsum(acc, csrow, ps_off, esc8):
                i = cs_n[acc]
                cs_n[acc] += 1
                nc.tensor.matmul(
                    cs_acc[acc], lhsT=sel8[:, :, 4 * csrow:4 * csrow + 4],
                    rhs=esc8[:, :, ps_off:ps_off + 512],
                    perf_mode=DR, start=(i == 0), stop=(i == cs_tot[acc] - 1),
                    skip_group_check=True)
                if cs_n[acc] == cs_tot[acc]:
                    # accumulator complete: evacuate + ship while PE runs on.
                    # cs_b finishes last — use ScalarE (free after the final
                    # exp, faster from PSUM) + the idle gpsimd DMA queue
                    sb, dr = cs_out[acc]
                    if acc == "a":
                        nc.vector.tensor_copy(out=sb, in_=cs_acc[acc])
                        nc.sync.dma_start(out=dr.ap(), in_=sb)
                    else:
                        nc.scalar.copy(out=sb, in_=cs_acc[acc])
                        nc.gpsimd.dma_start(out=dr.ap(), in_=sb)

            def queue_colsums(band, pair, esc8):
                for ps_off, acc, csrow in BAND_CS[band]:
                    if csrow == "d4":
                        csrow = 2 + (pair // 2)
                    cs_q.append((acc, csrow, ps_off, esc8))

            gidx = 0
            for band in BAND_ORDER:
                for rb in range(RT):
                    ps = mpp.tile([128, GW], f32, tag="ps")
                    if rb % 2 == 0:
                        esc_pair[0] = esc8p.tile([128, 2, GW], fp8e5,
                                                 tag="esc8", name="esc8")
                    esc8 = esc_pair[0]
                    fb = fbands[band]
                    # first group runs ct-outer so it is not stalled
                    # mid-group by the last-arriving DMA chunk
                    if band == BAND_ORDER[0] and rb == 0:
                        mm_order = [(kc2, ct) for ct in range(3)
                                    for kc2 in range(2)]
                    else:
                        mm_order = [(kc2, ct) for kc2 in range(2)
                                    for ct in range(3)]
                    for idx, (kc2, ct) in enumerate(mm_order):
                        cc = ct
                        if band == 2 and ct == 2:
                            cc = 2 + rb // 4
                        nc.tensor.matmul(
                            ps[:, 512 * ct:512 * ct + 512],
                            lhsT=fb0[:, rb // 4, 2 * kc2:2 * kc2 + 2,
                                     128 * (rb % 4):128 * (rb % 4) + 128],
                            rhs=fb[:, cc, 2 * kc2:2 * kc2 + 2],
                            perf_mode=DR,
                            start=(kc2 == 0),
                            stop=(kc2 == 1))
                        if cs_q:
                            one_colsum(*cs_q.pop(0))
                    # deferred colsums ride the PE queue behind these matmuls
                    while pend and pend[0][0] <= gidx:
                        _, b_, p_, e_ = pend.pop(0)
                        queue_colsums(b_, p_, e_)
                    slot = rb * NB + band
                    if band in (1, 2) and rb % 4 != 3:
                        # offload bands 1-2 rowsums to the idle DVE (reduces
                        # the fp8 exp tile in SBUF): the saturated ACT queue
                        # skips the accumulator read AND the psum group is
                        # released at exp end (not read end), unblocking PE
                        # sooner. Band 0 must stay on the exact ACT
                        # accumulator (it holds the dominant diagonal terms
                        # the host subtracts exactly); the last row tile
                        # stays on ACT so zacs completes with the final exp.
                        nc.scalar.activation(
                            out=esc8[:, rb % 2], in_=ps,
                            func=A.Exp, scale=TEMP_INV)
                        nc.vector.tensor_reduce(
                            out=zacs[:, slot:slot + 1],
                            in_=esc8[:, rb % 2], axis=X, op=Alu.add)
                    else:
                        nc.scalar.activation(
                            out=esc8[:, rb % 2], in_=ps,
                            func=A.Exp, scale=TEMP_INV,
                            accum_out=zacs[:, slot:slot + 1])
                    if rb % 2 == 1:
                        pend.append((gidx + 2, band, rb // 2, esc8))
                    gidx += 1
            while pend:
                _, b_, p_, e_ = pend.pop(0)
                queue_colsums(b_, p_, e_)
            while cs_q:
                one_colsum(*cs_q.pop(0))

        nc.sync.dma_start(out=zoutd.ap(), in_=zacs)

    nc.compile()
    return nc


def _get_program():
    global _prog_cache
    if _prog_cache is None:
        _prog_cache = _build_program()
    return _prog_cache


def _prep_inputs(features, labels):
    f8t = ml_dtypes.float8_e4m3
    f = np.asarray(features, dtype=np.float32)
    lab = np.asarray(labels).astype(np.int64)

    norm = np.maximum(np.sqrt((f * f).sum(axis=1, keepdims=True)), 1e-12)
    fn = f / norm                                   # [N, D] f32, unit rows
    f8 = fn.astype(f8t)                             # device values

    # fT8_full[p, kc, j] = f8[j, kc*128 + p]
    fT8_full = np.ascontiguousarray(
        f8.T.reshape(KC, 128, N).transpose(1, 0, 2))
    fT8_dbl = np.concatenate([fT8_full, fT8_full], axis=2)

    # select-ones weights: sel8[p, r, 4c + m] = (m == c), c in 0..3
    sel8 = np.zeros((128, 2, 16), np.float32)
    for c in range(4):
        sel8[:, :, 4 * c + c] = 1.0
    sel8 = sel8.astype(ml_dtypes.float8_e5m2)

    band_bounds = ((0, 1536), (1536, 3072), (3072, 5120))
    in_maps = []
    for k in range(NCORES):
        loc = fT8_dbl[:, :, k * RPC:k * RPC + LC].copy()
        if k >= 4:
            # swap the two 512-col halves of the d=4 block
            loc[:, :, 4096:5120] = np.concatenate(
                [loc[:, :, 4608:5120], loc[:, :, 4096:4608]], axis=2)
        # chunk-major packing: each 512-col chunk contiguous per partition
        parts = []
        for lo, hi in band_bounds:
            for c0 in range(lo, hi, 512):
                parts.append(loc[:, :, c0:c0 + 512].reshape(128, -1))
        packed = np.concatenate(parts, axis=1)
        in_maps.append({
            "fT8": np.ascontiguousarray(packed),
            "sel8": sel8,
        })

    # host-side O(N*D) terms, f32 like the reference
    oh = np.stack([lab == 0, lab == 1], axis=1).astype(np.float32)
    counts = oh.sum(axis=0)
    npos = (counts[lab] - 1).astype(np.float32)     # positives excl. self
    g = fn.T @ oh                                   # [D, 2] class sums
    rddot = (fn @ g)[np.arange(N), lab]             # f_i . g_{lab_i}
    sii = (fn * fn).sum(axis=1)                     # ~1.0
    possum = TEMP_INV * (rddot - sii)               # sum_{j in pos, j!=i} s_ij
    # device rowsums now include the unmasked diagonal exp(10 * sum f8_i^2);
    # replicate it exactly on the host (fp8 squares are f32-exact, ACT exp
    # is within 2 ULP of np.exp) and subtract
    f8f = f8.astype(np.float32)
    diag_exp = np.exp(TEMP_INV * (f8f * f8f).sum(axis=1).astype(np.float64))
    return in_maps, npos, possum, diag_exp


def _colsum_global_cols(k):
    """Global column index for each of core k's CC colsum outputs."""
    t = np.arange(CC)
    g = (k * RPC + 1024 + t) % N
    if k >= 4:
        a = ((k + 4) % 8) * RPC
        tl = t[(t >= 3072) & (t < 3584)]
        tr = t[t >= 3584]
        g = g.copy()
        g[tl] = a + 512 + (tl - 3072)
        g[tr] = a + (tr - 3584)
    return g


def _run(inputs, trace=False, trace_kwargs=None):
    from concourse.bass_utils import run_bass_kernel_spmd

    nc = _get_program()
    in_maps, npos, possum, diag_exp = _prep_inputs(inputs["features"],
                                                   inputs["labels"])
    res = run_bass_kernel_spmd(nc, in_maps, core_ids=list(range(NCORES)),
                               trace=trace, **(trace_kwargs or {}))
    Z = np.zeros((N,), np.float64)
    for k in range(NCORES):
        # zout[p, rb*NB + band]: band-partial rowsums of row k*RPC + rb*128+p
        zk = res.results[k]["zout"].reshape(128, RT, NB).sum(axis=2)
        Z[k * RPC:(k + 1) * RPC] = zk.T.reshape(RPC)
    for k in range(NCORES):
        cs = np.concatenate(
            [res.results[k]["cout_a"].reshape(2048),
             res.results[k]["cout_b"].reshape(2048)]).astype(np.float64)
        np.add.at(Z, _colsum_global_cols(k), cs)
    Z -= diag_exp
    lnZ = np.log(Z)
    per_row = (npos * lnZ - possum) / (npos + EPS)
    loss = np.float32(per_row.mean())
    return loss, res


def kernel(**inputs) -> np.ndarray:
    loss, _ = _run(inputs, trace=False)
    return np.asarray(loss, dtype=np.float32)


# revision 47
# speedup vs baseline: 1.0211x; 1.0211x over previous
"""ContrastiveLoss Trainium2 kernel (8 NeuronCores, SPMD row-sharded,
symmetry-exploiting).

Math (reference):
    f = features / ||features||_row            (L2 normalize)
    s_ij = (f_i . f_j) / T,  T = 0.1
    Z_i = sum_{j != i} exp(s_ij)
    per_row_i = (npos_i * ln(Z_i) - sum_{j in pos, j != i} s_ij) / (npos_i + eps)
    loss = mean(per_row)

Device computes ONLY the O(N^2) part: partial sums of exp(10 * f_i . f_j).
Each unordered (i, j) pair is computed exactly once across the 8 cores
(N^2/2 total work): core k owns rows [k*1024, (k+1)*1024) and computes
local columns [0, 4608) of its rotated column space:
  [0, 1024)     own (diagonal) block, self-terms masked
  [1024, 4096)  the next 3 row-blocks (d = 1, 2, 3)
  [4096, 4608)  a checkerboard half of the d = 4 block: row-half rb0-3 gets
                one 512-column half, rb4-7 the other; cores 4-7 receive the
                two halves host-swapped so the 4 quarters of each d=4 block
                pair are covered exactly once globally.
Row sums (ACT accum) give Z contributions for the core's own rows; column
sums (DoubleRow select-ones matmuls over fp8e5 exp tiles, accumulated
in-place in one PSUM bank) give the contributions to the partner rows,
which the host scatters back. All O(N*D) math (normalize, transpose,
class sums, final assembly) runs on the host in f32.
"""

import numpy as np
import ml_dtypes

TEMP_INV = 10.0  # 1/temperature
EPS = 1e-8
N, D, NCORES = 8192, 512, 8
RPC = N // NCORES        # 1024 rows per core
RT = RPC // 128          # 8 row tiles (128 rows) per core
KC = D // 128            # 4 contraction chunks of 128
GW = 1536                # psum group width (3 banks)
NB = 3                   # bands (psum groups) per row tile
LC = 5120                # local columns resident in SBUF
CC = 4096                # columns with column-sum output (local 1024..5120)
WARMUP_MMS = 40          # dummy matmuls to keep the PE busy (and the HAM
                         # clock gate warming) until the input DMA lands

# bands cover local columns [0,1536), [1536,3072), [3072,5120); band 2's
# third matmul tile is the d4 half: local col 4096 + 512*(rb//4)
BAND_ORDER = [0, 1, 2]  # band 0 first: it holds the lhsT columns
BAND_W = {0: 1536, 1: 1536, 2: 2048}
# colsum chunks per band: (psum offset, accumulator a/b, accumulator row)
BAND_CS = [((1024, "a", 0),),
           ((0, "a", 1), (512, "a", 2), (1024, "a", 3)),
           ((0, "b", 0), (512, "b", 1), (1024, "b", "d4"))]
NCS_A = 16               # colsum matmuls into accumulator a (bands 0-1)
NCS_B = 12               # colsum matmuls into accumulator b (band 2 + d4)

_prog_cache = None


def _build_program():
    import concourse.bacc as bacc
    import concourse.tile as tile
    from concourse import mybir

    f32, bf16 = mybir.dt.float32, mybir.dt.bfloat16
    fp8, fp8e5 = mybir.dt.float8e4, mybir.dt.float8e5
    A = mybir.ActivationFunctionType
    Alu = mybir.AluOpType
    X = mybir.AxisListType.X
    DR = mybir.MatmulPerfMode.DoubleRow

    nc = bacc.Bacc("TRN2", target_bir_lowering=False, debug=False,
                   num_devices=NCORES)

    # chunk-major packed transposed features, [128, KC*LC] fp8e4; each
    # 512-col chunk is one contiguous 2 KiB run per partition
    fT8d = nc.dram_tensor("fT8", [128, KC * LC], fp8, kind="ExternalInput")
    seld = nc.dram_tensor("sel8", [128, 2, 16], fp8e5, kind="ExternalInput")
    zoutd = nc.dram_tensor("zout", [128, RT * NB], f32,
                           kind="ExternalOutput")
    coutad = nc.dram_tensor("cout_a", [4, 512], f32, kind="ExternalOutput")
    coutbd = nc.dram_tensor("cout_b", [4, 512], f32, kind="ExternalOutput")

    from contextlib import ExitStack

    with tile.TileContext(nc) as tc, ExitStack() as ctx:
        singles = ctx.enter_context(tc.tile_pool(name="singles", bufs=1))
        esc8p = ctx.enter_context(tc.tile_pool(name="esc8p", bufs=6))

        # chunk-major band tiles: [128, chunk, KC, 512] so each 512-col DMA
        # chunk is one contiguous 2 KiB run per partition (1 descriptor)
        fb0 = singles.tile([128, 3, KC, 512], fp8, tag="fb0", name="fb0")
        fb1 = singles.tile([128, 3, KC, 512], fp8, tag="fb1", name="fb1")
        fb2 = singles.tile([128, 4, KC, 512], fp8, tag="fb2", name="fb2")
        fbands = [fb0, fb1, fb2]
        sel8 = singles.tile([128, 2, 16], fp8e5, tag="sel8")
        warm = singles.tile([128, 256], fp8, tag="warm")
        dumm = singles.tile([128, 256], bf16, tag="dumm")
        zacs = singles.tile([128, RT * NB], f32, tag="zacs")
        couts_a = singles.tile([4, 512], f32, tag="couts_a")
        couts_b = singles.tile([4, 512], f32, tag="couts_b")

        nc.vector.memset(warm, 0.0)

        # input DMA, chunked 512 cols at a time in compute order; band 0's
        # three chunks go out first on three queues in parallel — they gate
        # the first psum group, so they get the full HBM bandwidth. Bands
        # 1-2 are issued from gpsimd only after a tiny gate-copy that reads
        # one element of each band-0 chunk (i.e. after band 0 has landed),
        # so their transfers don't steal bandwidth from band 0.
        gate = singles.tile([1, 3, 1], fp8, tag="gate")
        nc.gpsimd.dma_start(out=sel8, in_=seld.ap())
        b0_queues = [nc.sync, nc.gpsimd, nc.scalar]
        o = 0
        for c in range(3):
            b0_queues[c].dma_start(
                out=fb0[:, c],
                in_=fT8d.ap()[:, o:o + KC * 512].rearrange(
                    "p (k w) -> p k w", k=KC))
            o += KC * 512
        nc.gpsimd.tensor_copy(out=gate, in_=fb0[0:1, :, 0, 0:1])
        for b in (1, 2):
            for c in range(BAND_W[b] // 512):
                nc.gpsimd.dma_start(
                    out=fbands[b][:, c],
                    in_=fT8d.ap()[:, o:o + KC * 512].rearrange(
                        "p (k w) -> p k w", k=KC))
                o += KC * 512

        with tc.tile_pool(name="mps", bufs=2, space="PSUM") as mpp, \
                tc.tile_pool(name="cps", bufs=1, space="PSUM") as cpp:
            # two accumulator banks for the column sums: row r of cs_a/cs_b
            # accumulates column-sum chunk r / 4+r (via select-ones weights);
            # cs_a (bands 0-1) completes early and is flushed mid-kernel
            cs_a = cpp.tile([4, 512], f32, tag="cs_a")
            cs_b = cpp.tile([4, 512], f32, tag="cs_b")
            cs_acc = {"a": cs_a, "b": cs_b}
            cs_n = {"a": 0, "b": 0}
            cs_tot = {"a": NCS_A, "b": NCS_B}
            cs_out = {"a": (couts_a, coutad), "b": (couts_b, coutbd)}

            # warm the PE + trigger the ACT exp table load while DMA streams
            wps = mpp.tile([128, GW], f32, tag="ps")
            for w in range(WARMUP_MMS):
                nc.tensor.matmul(wps[:, :128], lhsT=warm[:, :128],
                                 rhs=warm[:, 128:], start=True, stop=True)
            nc.scalar.activation(out=dumm, in_=warm, func=A.Exp,
                                 scale=TEMP_INV)

            esc_pair = [None]
            pend = []  # deferred colsums: (ready_gidx, band, pair, esc tile)
            cs_q = []  # colsum closures, popped one per main matmul slot

            def one_colsum(acc, csrow, ps_off, esc8):
                i = cs_n[acc]
                cs_n[acc] += 1
                nc.tensor.matmul(
                    cs_acc[acc], lhsT=sel8[:, :, 4 * csrow:4 * csrow + 4],
                    rhs=esc8[:, :, ps_off:ps_off + 512],
                    perf_mode=DR, start=(i == 0), stop=(i == cs_tot[acc] - 1),
                    skip_group_check=True)
                if cs_n[acc] == cs_tot[acc]:
                    # accumulator complete: evacuate + ship while PE runs on.
                    # cs_b finishes last — use ScalarE (free after the final
                    # exp, faster from PSUM) + the idle gpsimd DMA queue
                    sb, dr = cs_out[acc]
                    if acc == "a":
                        nc.vector.tensor_copy(out=sb, in_=cs_acc[acc])
                        nc.sync.dma_start(out=dr.ap(), in_=sb)
                    else:
                        nc.scalar.copy(out=sb, in_=cs_acc[acc])
                        nc.gpsimd.dma_start(out=dr.ap(), in_=sb)

            def queue_colsums(band, pair, esc8):
                for ps_off, acc, csrow in BAND_CS[band]:
                    if csrow == "d4":
                        csrow = 2 + (pair // 2)
                    cs_q.append((acc, csrow, ps_off, esc8))

            gidx = 0
            for band in BAND_ORDER:
                for rb in range(RT):
                    ps = mpp.tile([128, GW], f32, tag="ps")
                    if rb % 2 == 0:
                        esc_pair[0] = esc8p.tile([128, 2, GW], fp8e5,
                                                 tag="esc8", name="esc8")
                    esc8 = esc_pair[0]
                    fb = fbands[band]
                    # first group runs ct-outer so it is not stalled
                    # mid-group by the last-arriving DMA chunk
                    if band == BAND_ORDER[0] and rb == 0:
                        mm_order = [(kc2, ct) for ct in range(3)
                                    for kc2 in range(2)]
                    else:
                        mm_order = [(kc2, ct) for kc2 in range(2)
                                    for ct in range(3)]
                    for idx, (kc2, ct) in enumerate(mm_order):
                        cc = ct
                        if band == 2 and ct == 2:
                            cc = 2 + rb // 4
                        nc.tensor.matmul(
                            ps[:, 512 * ct:512 * ct + 512],
                            lhsT=fb0[:, rb // 4, 2 * kc2:2 * kc2 + 2,
                                     128 * (rb % 4):128 * (rb % 4) + 128],
                            rhs=fb[:, cc, 2 * kc2:2 * kc2 + 2],
                            perf_mode=DR,
                            start=(kc2 == 0),
                            stop=(kc2 == 1))
                        if cs_q:
                            one_colsum(*cs_q.pop(0))
                    # deferred colsums ride the PE queue behind these matmuls
                    while pend and pend[0][0] <= gidx:
                        _, b_, p_, e_ = pend.pop(0)
                        queue_colsums(b_, p_, e_)
                    slot = rb * NB + band
                    if band in (1, 2) and not (band == 2 and rb == RT - 1):
                        # offload bands 1-2 rowsums to the idle DVE (reduces
                        # the fp8 exp tile in SBUF): the saturated ACT queue
                        # skips the accumulator read AND the psum group is
                        # released at exp end (not read end), unblocking PE
                        # sooner. Band 0 must stay on the exact ACT
                        # accumulator (it holds the dominant diagonal terms
                        # the host subtracts exactly); the last row tile
                        # stays on ACT so zacs completes with the final exp.
                        nc.scalar.activation(
                            out=esc8[:, rb % 2], in_=ps,
                            func=A.Exp, scale=TEMP_INV)
                        nc.vector.tensor_reduce(
                            out=zacs[:, slot:slot + 1],
                            in_=esc8[:, rb % 2], axis=X, op=Alu.add)
                    else:
                        nc.scalar.activation(
                            out=esc8[:, rb % 2], in_=ps,
                            func=A.Exp, scale=TEMP_INV,
                            accum_out=zacs[:, slot:slot + 1])
                    if rb % 2 == 1:
                        pend.append((gidx + 2, band, rb // 2, esc8))
                    gidx += 1
            while pend:
                _, b_, p_, e_ = pend.pop(0)
                queue_colsums(b_, p_, e_)
            while cs_q:
                one_colsum(*cs_q.pop(0))

        nc.sync.dma_start(out=zoutd.ap(), in_=zacs)

    nc.compile()
    return nc


def _get_program():
    global _prog_cache
    if _prog_cache is None:
        _prog_cache = _build_program()
    return _prog_cache


def _prep_inputs(features, labels):
    f8t = ml_dtypes.float8_e4m3
    f = np.asarray(features, dtype=np.float32)
    lab = np.asarray(labels).astype(np.int64)

    norm = np.maximum(np.sqrt((f * f).sum(axis=1, keepdims=True)), 1e-12)
    fn = f / norm                                   # [N, D] f32, unit rows
    f8 = fn.astype(f8t)                             # device values

    # fT8_full[p, kc, j] = f8[j, kc*128 + p]
    fT8_full = np.ascontiguousarray(
        f8.T.reshape(KC, 128, N).transpose(1, 0, 2))
    fT8_dbl = np.concatenate([fT8_full, fT8_full], axis=2)

    # select-ones weights: sel8[p, r, 4c + m] = (m == c), c in 0..3
    sel8 = np.zeros((128, 2, 16), np.float32)
    for c in range(4):
        sel8[:, :, 4 * c + c] = 1.0
    sel8 = sel8.astype(ml_dtypes.float8_e5m2)

    band_bounds = ((0, 1536), (1536, 3072), (3072, 5120))
    in_maps = []
    for k in range(NCORES):
        loc = fT8_dbl[:, :, k * RPC:k * RPC + LC].copy()
        if k >= 4:
            # swap the two 512-col halves of the d=4 block
            loc[:, :, 4096:5120] = np.concatenate(
                [loc[:, :, 4608:5120], loc[:, :, 4096:4608]], axis=2)
        # chunk-major packing: each 512-col chunk contiguous per partition
        parts = []
        for lo, hi in band_bounds:
            for c0 in range(lo, hi, 512):
                parts.append(loc[:, :, c0:c0 + 512].reshape(128, -1))
        packed = np.concatenate(parts, axis=1)
        in_maps.append({
            "fT8": np.ascontiguousarray(packed),
            "sel8": sel8,
        })

    # host-side O(N*D) terms, f32 like the reference
    oh = np.stack([lab == 0, lab == 1], axis=1).astype(np.float32)
    counts = oh.sum(axis=0)
    npos = (counts[lab] - 1).astype(np.float32)     # positives excl. self
    g = fn.T @ oh                                   # [D, 2] class sums
    rddot = (fn @ g)[np.arange(N), lab]             # f_i . g_{lab_i}
    sii = (fn * fn).sum(axis=1)                     # ~1.0
    possum = TEMP_INV * (rddot - sii)               # sum_{j in pos, j!=i} s_ij
    # device rowsums now include the unmasked diagonal exp(10 * sum f8_i^2);
    # replicate it exactly on the host (fp8 squares are f32-exact, ACT exp
    # is within 2 ULP of np.exp) and subtract
    f8f = f8.astype(np.float32)
    diag_exp = np.exp(TEMP_INV * (f8f * f8f).sum(axis=1).astype(np.float64))
    return in_maps, npos, possum, diag_exp


def _colsum_global_cols(k):
    """Global column index for each of core k's CC colsum outputs."""
    t = np.arange(CC)
    g = (k * RPC + 1024 + t) % N
    if k >= 4:
        a = ((k + 4) % 8) * RPC
        tl = t[(t >= 3072) & (t < 3584)]
        tr = t[t >= 3584]
        g = g.copy()
        g[tl] = a + 512 + (tl - 3072)
        g[tr] = a + (tr - 3584)
    return g


def _run(inputs, trace=False, trace_kwargs=None):
    from concourse.bass_utils import run_bass_kernel_spmd

    nc = _get_program()
    in_maps, npos, possum, diag_exp = _prep_inputs(inputs["features"],
                                                   inputs["labels"])
    res = run_bass_kernel_spmd(nc, in_maps, core_ids=list(range(NCORES)),
                               trace=trace, **(trace_kwargs or {}))
    Z = np.zeros((N,), np.float64)
    for k in range(NCORES):
        # zout[p, rb*NB + band]: band-partial rowsums of row k*RPC + rb*128+p
        zk = res.results[k]["zout"].reshape(128, RT, NB).sum(axis=2)
        Z[k * RPC:(k + 1) * RPC] = zk.T.reshape(RPC)
    for k in range(NCORES):
        cs = np.concatenate(
            [res.results[k]["cout_a"].reshape(2048),
             res.results[k]["cout_b"].reshape(2048)]).astype(np.float64)
        np.add.at(Z, _colsum_global_cols(k), cs)
    Z -= diag_exp
    lnZ = np.log(Z)
    per_row = (npos * lnZ - possum) / (npos + EPS)
    loss = np.float32(per_row.mean())
    return loss, res


def kernel(**inputs) -> np.ndarray:
    loss, _ = _run(inputs, trace=False)
    return np.asarray(loss, dtype=np.float32)
